# revision 39
# baseline (speedup 1.0000x reference)
"""HadamardNorm kernel for Trainium2 (8 NeuronCores, pure data parallel).

Computes y = LeakyReLU_{0.1}( FWHT_4096(x) / sqrt(4096) ) row-wise on
x of shape (4, 4096, 4096) fp32.

Math: Sylvester FWHT_4096 = H32 (x) H128 with row element
idx = J*128 + c (J in 32, c in 128):
  Y[j', c'] = sum_{J,c} H32[j',J] H128[c',c] X[J,c]

Per-core (2048 rows = 16 tiles of 128 rows; tile row = r4*32 + r32).
The host pre-permutes x into x_dev[t, p=(r4,J), f=(r32,c)] bf16 and
un-permutes y_dev[t, p=c', f=(r32,r4,j')] back, so every DMA is a
contiguous [128 part x 2048] block (4KB packets at full engine rate).

  IN    [(r4,J) part, (r32,c) free]        <- SWDGE DMA, 2 half-tile DMAs
  MM1   lhsT = IN chunk r32 [., c], rhs = W1 = I4 (x) H32   (per chunk)
        -> PSUM Z_r32[c, (r4',j')]         (contracts J)
  ZB    DVE copy PSUM->SBUF, cast bf16
  MM2   lhsT = H128 (stationary, reused), rhs = ZB [c, 512]
        -> PSUM Y[c', (r32, r4', j')]      (contracts c, 512-wide stream)
  OUT   ACT Prelu(scale=1/64, alpha=0.1) PSUM -> SBUF bf16
  y     <- HWDGE DMA store, 2 half-tile DMAs; host upcasts to fp32.

No transposes: MM1 uses the data as the stationary (lhsT) operand which
swaps the partition dim to c; MM2 streams the data against a stationary
H128, swapping the partition dim to c'.
"""

import numpy as np

import concourse.bass as bass
import concourse.mybir as mybir
import concourse.tile as tile
from concourse import bacc
from concourse.bass_utils import run_bass_kernel_spmd

N_CORES = 8
D = 4096
ROWS_TOTAL = 4 * 4096                   # 16384 rows of 4096
ROWS_PER_CORE = ROWS_TOTAL // N_CORES   # 2048
NT = ROWS_PER_CORE // 128               # 16 tiles of 128 rows per core

F32 = mybir.dt.float32
BF16 = mybir.dt.bfloat16

GROUPS = 4          # chunk groups per tile (8 r32 chunks each)
GSZ = 32 // GROUPS


def _hadamard(n: int) -> np.ndarray:
    h = np.array([[1.0]], dtype=np.float32)
    while h.shape[0] < n:
        h = np.block([[h, h], [h, -h]])
    assert h.shape[0] == n
    return h.astype(np.float32)


def _build_nc():
    import ml_dtypes

    W1 = np.kron(np.eye(4, dtype=np.float32), _hadamard(32))   # [128,128]
    H128 = _hadamard(128)                                      # [128,128]

    nc = bacc.Bacc("TRN2", target_bir_lowering=False, debug=False,
                   num_devices=N_CORES)

    x = nc.dram_tensor("x", [NT, 128, 4096], BF16, kind="ExternalInput")
    y = nc.dram_tensor("y", [NT, 128, 4096], BF16, kind="ExternalOutput")

    w1_d = nc.inline_tensor(W1.astype(ml_dtypes.bfloat16), "w1c")
    h128_d = nc.inline_tensor(H128.astype(ml_dtypes.bfloat16), "h128c")

    with tile.TileContext(nc) as tc:
        with (
            tc.tile_pool(name="wpool", bufs=1) as wpool,
            tc.tile_pool(name="inp", bufs=3) as inp,
            tc.tile_pool(name="ps1p", bufs=2, space="PSUM") as ps1p,
            tc.tile_pool(name="zbp", bufs=3) as zbp,
            tc.tile_pool(name="ps2p", bufs=2, space="PSUM") as ps2p,
            tc.tile_pool(name="outp", bufs=4) as outp,
        ):
            w1 = wpool.tile([128, 128], BF16, tag="w1")
            h128 = wpool.tile([128, 128], BF16, tag="h128")
            nc.sync.dma_start(w1[:], w1_d[:])
            nc.sync.dma_start(h128[:], h128_d[:])

            for t in range(NT):
                tin = inp.tile([128, 4096], BF16, tag="tin")
                for h in range(2):
                    nc.gpsimd.dma_start(
                        tin[:, h * 2048:(h + 1) * 2048],
                        x[t][:, h * 2048:(h + 1) * 2048])
                tout = outp.tile([128, 4096], BF16, tag="tout")
                for g in range(GROUPS):
                    ps1 = ps1p.tile([128, GSZ * 128], F32, tag="ps1")
                    for k in range(GSZ):
                        r32 = g * GSZ + k
                        nc.tensor.matmul(
                            ps1[:, k * 128:(k + 1) * 128],
                            tin[:, r32 * 128:(r32 + 1) * 128],
                            w1[:], start=True, stop=True)
                    zb = zbp.tile([128, GSZ * 128], BF16, tag="zb")
                    nc.vector.tensor_copy(zb[:], ps1[:])
                    ps2 = ps2p.tile([128, GSZ * 128], F32, tag="ps2")
                    for m in range(2):
                        nc.tensor.matmul(
                            ps2[:, m * 512:(m + 1) * 512],
                            h128[:],
                            zb[:, m * 512:(m + 1) * 512],
                            start=True, stop=True)
                    nc.scalar.activation(
                        tout[:, g * GSZ * 128:(g + 1) * GSZ * 128],
                        ps2[:],
                        mybir.ActivationFunctionType.Prelu,
                        bias=0.0, scale=1.0 / 64.0, alpha=0.1)
                    if g == 1 or g == 3:
                        h = g // 2
                        nc.sync.dma_start(
                            y[t][:, h * 2048:(h + 1) * 2048],
                            tout[:, h * 2048:(h + 1) * 2048])
    nc.finalize()
    return nc


_NC_CACHE = {}


def _get_nc():
    if "nc" not in _NC_CACHE:
        _NC_CACHE["nc"] = _build_nc()
    return _NC_CACHE["nc"]


def run(x: np.ndarray, trace: bool = False):
    """Returns (y, BassKernelResults)."""
    import ml_dtypes

    x = np.ascontiguousarray(x, dtype=np.float32)
    xb = x.reshape(-1, D).astype(ml_dtypes.bfloat16)
    shards = []
    for c in range(N_CORES):
        v = xb[c * ROWS_PER_CORE:(c + 1) * ROWS_PER_CORE]
        # rows [t, r4, r32, J, c] -> [t, (r4 J), (r32 c)]
        v = v.reshape(NT, 4, 32, 32, 128).transpose(0, 1, 3, 2, 4)
        shards.append(np.ascontiguousarray(v).reshape(NT, 128, 4096))
    nc = _get_nc()
    res = run_bass_kernel_spmd(
        nc, [{"x": s} for s in shards], core_ids=list(range(N_CORES)),
        trace=trace)
    outs = []
    for r in res.results:
        # y_dev [t, c', (r32, r4, j')] -> rows [t, r4, r32, j', c'].
        # Permute in bf16 (dtype-agnostic strided copy), then upcast
        # contiguously — astype on a strided bf16 view is very slow.
        v = np.asarray(r["y"]).reshape(NT, 128, 32, 4, 32)
        v = np.ascontiguousarray(v.transpose(0, 3, 2, 4, 1))
        outs.append(v.astype(np.float32).reshape(ROWS_PER_CORE, D))
    out = np.concatenate(outs, axis=0)
    return out.reshape(x.shape), res


def kernel(x: np.ndarray) -> np.ndarray:
    out, _ = run(x, trace=False)
    return out


# revision 40
# speedup vs baseline: 1.0328x; 1.0328x over previous
"""HadamardNorm kernel for Trainium2 (8 NeuronCores, pure data parallel).

Computes y = LeakyReLU_{0.1}( FWHT_4096(x) / sqrt(4096) ) row-wise on
x of shape (4, 4096, 4096) fp32.

Math: Sylvester FWHT_4096 = H32 (x) H128 with row element
idx = J*128 + c (J in 32, c in 128):
  Y[j', c'] = sum_{J,c} H32[j',J] H128[c',c] X[J,c]

Per-core (2048 rows = 16 tiles of 128 rows; tile row = r4*32 + r32).
The host pre-permutes x into x_dev[t, p=(r4,J), f=(r32,c)] bf16 and
un-permutes y_dev[t, p=c', f=(r32,r4,j')] back, so every DMA is a
contiguous [128 part x 2048] block (4KB packets at full engine rate).

  IN    [(r4,J) part, (r32,c) free]        <- SWDGE DMA, 2 half-tile DMAs
  MM1   lhsT = IN chunk r32 [., c], rhs = W1 = I4 (x) H32   (per chunk)
        -> PSUM Z_r32[c, (r4',j')]         (contracts J)
  ZB    DVE copy PSUM->SBUF, cast bf16
  MM2   lhsT = H128 (stationary, reused), rhs = ZB [c, 512]
        -> PSUM Y[c', (r32, r4', j')]      (contracts c, 512-wide stream)
  OUT   ACT Prelu(scale=1/64, alpha=0.1) PSUM -> SBUF bf16
  y     <- HWDGE DMA store, 2 half-tile DMAs; host upcasts to fp32.

No transposes: MM1 uses the data as the stationary (lhsT) operand which
swaps the partition dim to c; MM2 streams the data against a stationary
H128, swapping the partition dim to c'.
"""

import numpy as np

import concourse.bass as bass
import concourse.mybir as mybir
import concourse.tile as tile
from concourse import bacc
from concourse.bass_utils import run_bass_kernel_spmd

N_CORES = 8
D = 4096
ROWS_TOTAL = 4 * 4096                   # 16384 rows of 4096
ROWS_PER_CORE = ROWS_TOTAL // N_CORES   # 2048
NT = ROWS_PER_CORE // 128               # 16 tiles of 128 rows per core

F32 = mybir.dt.float32
BF16 = mybir.dt.bfloat16

GROUPS = 4          # chunk groups per tile (8 r32 chunks each)
GSZ = 32 // GROUPS


def _hadamard(n: int) -> np.ndarray:
    h = np.array([[1.0]], dtype=np.float32)
    while h.shape[0] < n:
        h = np.block([[h, h], [h, -h]])
    assert h.shape[0] == n
    return h.astype(np.float32)


def _build_nc():
    import ml_dtypes

    W1 = np.kron(np.eye(4, dtype=np.float32), _hadamard(32))   # [128,128]
    H128 = _hadamard(128)                                      # [128,128]

    nc = bacc.Bacc("TRN2", target_bir_lowering=False, debug=False,
                   num_devices=N_CORES)

    x = nc.dram_tensor("x", [NT, 128, 4096], BF16, kind="ExternalInput")
    y = nc.dram_tensor("y", [NT, 128, 4096], BF16, kind="ExternalOutput")

    w1_d = nc.inline_tensor(W1.astype(ml_dtypes.bfloat16), "w1c")
    h128_d = nc.inline_tensor(H128.astype(ml_dtypes.bfloat16), "h128c")

    with tile.TileContext(nc) as tc:
        with (
            tc.tile_pool(name="wpool", bufs=1) as wpool,
            tc.tile_pool(name="inp", bufs=3) as inp,
            tc.tile_pool(name="ps1p", bufs=2, space="PSUM") as ps1p,
            tc.tile_pool(name="zbp", bufs=3) as zbp,
            tc.tile_pool(name="ps2p", bufs=2, space="PSUM") as ps2p,
            tc.tile_pool(name="outp", bufs=3) as outp,
        ):
            w1 = wpool.tile([128, 128], BF16, tag="w1")
            h128 = wpool.tile([128, 128], BF16, tag="h128")
            nc.sync.dma_start(w1[:], w1_d[:])
            nc.sync.dma_start(h128[:], h128_d[:])

            for t in range(NT):
                tin = inp.tile([128, 4096], BF16, tag="tin")
                for h in range(2):
                    nc.gpsimd.dma_start(
                        tin[:, h * 2048:(h + 1) * 2048],
                        x[t][:, h * 2048:(h + 1) * 2048])
                tout = outp.tile([128, 4096], BF16, tag="tout")
                for g in range(GROUPS):
                    ps1 = ps1p.tile([128, GSZ * 128], F32, tag="ps1")
                    for k in range(GSZ):
                        r32 = g * GSZ + k
                        nc.tensor.matmul(
                            ps1[:, k * 128:(k + 1) * 128],
                            tin[:, r32 * 128:(r32 + 1) * 128],
                            w1[:], start=True, stop=True)
                    zb = zbp.tile([128, GSZ * 128], BF16, tag="zb")
                    nc.vector.tensor_copy(zb[:], ps1[:])
                    ps2 = ps2p.tile([128, GSZ * 128], F32, tag="ps2")
                    for m in range(2):
                        nc.tensor.matmul(
                            ps2[:, m * 512:(m + 1) * 512],
                            h128[:],
                            zb[:, m * 512:(m + 1) * 512],
                            start=True, stop=True)
                    nc.scalar.activation(
                        tout[:, g * GSZ * 128:(g + 1) * GSZ * 128],
                        ps2[:],
                        mybir.ActivationFunctionType.Prelu,
                        bias=0.0, scale=1.0 / 64.0, alpha=0.1)
                    if g == 1 or g == 3:
                        h = g // 2
                        nc.sync.dma_start(
                            y[t][:, h * 2048:(h + 1) * 2048],
                            tout[:, h * 2048:(h + 1) * 2048])
    nc.finalize()
    return nc


_NC_CACHE = {}


def _get_nc():
    if "nc" not in _NC_CACHE:
        _NC_CACHE["nc"] = _build_nc()
    return _NC_CACHE["nc"]


def run(x: np.ndarray, trace: bool = False):
    """Returns (y, BassKernelResults)."""
    import ml_dtypes

    x = np.ascontiguousarray(x, dtype=np.float32)
    xb = x.reshape(-1, D).astype(ml_dtypes.bfloat16)
    shards = []
    for c in range(N_CORES):
        v = xb[c * ROWS_PER_CORE:(c + 1) * ROWS_PER_CORE]
        # rows [t, r4, r32, J, c] -> [t, (r4 J), (r32 c)]
        v = v.reshape(NT, 4, 32, 32, 128).transpose(0, 1, 3, 2, 4)
        shards.append(np.ascontiguousarray(v).reshape(NT, 128, 4096))
    nc = _get_nc()
    res = run_bass_kernel_spmd(
        nc, [{"x": s} for s in shards], core_ids=list(range(N_CORES)),
        trace=trace)
    outs = []
    for r in res.results:
        # y_dev [t, c', (r32, r4, j')] -> rows [t, r4, r32, j', c'].
        # Permute in bf16 (dtype-agnostic strided copy), then upcast
        # contiguously — astype on a strided bf16 view is very slow.
        v = np.asarray(r["y"]).reshape(NT, 128, 32, 4, 32)
        v = np.ascontiguousarray(v.transpose(0, 3, 2, 4, 1))
        outs.append(v.astype(np.float32).reshape(ROWS_PER_CORE, D))
    out = np.concatenate(outs, axis=0)
    return out.reshape(x.shape), res


def kernel(x: np.ndarray) -> np.ndarray:
    out, _ = run(x, trace=False)
    return out


# revision 44
# speedup vs baseline: 1.0676x; 1.0336x over previous
"""HadamardNorm kernel for Trainium2 (8 NeuronCores, pure data parallel).

Computes y = LeakyReLU_{0.1}( FWHT_4096(x) / sqrt(4096) ) row-wise on
x of shape (4, 4096, 4096) fp32.

Math: Sylvester FWHT_4096 = H32 (x) H128 with row element
idx = J*128 + c (J in 32, c in 128):
  Y[j', c'] = sum_{J,c} H32[j',J] H128[c',c] X[J,c]

Per-core (2048 rows = 16 tiles of 128 rows; tile row = r4*32 + r32).
The host pre-permutes x into x_dev[t, p=(r4,J), f=(r32,c)] bf16 and
un-permutes y_dev[t, p=c', f=(r32,r4,j')] back, so every DMA is a
contiguous [128 part x 2048] block (4KB packets at full engine rate).

  IN    [(r4,J) part, (r32,c) free]        <- SWDGE DMA, 2 half-tile DMAs
  MM1   lhsT = IN chunk r32 [., c], rhs = W1 = I4 (x) H32   (per chunk)
        -> PSUM Z_r32[c, (r4',j')]         (contracts J)
  ZB    DVE copy PSUM->SBUF, cast bf16
  MM2   lhsT = H128 (stationary, reused), rhs = ZB [c, 512]
        -> PSUM Y[c', (r32, r4', j')]      (contracts c, 512-wide stream)
  OUT   ACT Prelu(scale=1/64, alpha=0.1) PSUM -> SBUF bf16
  y     <- HWDGE DMA store, 2 half-tile DMAs; host upcasts to fp32.

No transposes: MM1 uses the data as the stationary (lhsT) operand which
swaps the partition dim to c; MM2 streams the data against a stationary
H128, swapping the partition dim to c'.
"""

import numpy as np

import concourse.bass as bass
import concourse.mybir as mybir
import concourse.tile as tile
from concourse import bacc
from concourse.bass_utils import run_bass_kernel_spmd

N_CORES = 8
D = 4096
ROWS_TOTAL = 4 * 4096                   # 16384 rows of 4096
ROWS_PER_CORE = ROWS_TOTAL // N_CORES   # 2048
NT = ROWS_PER_CORE // 128               # 16 tiles of 128 rows per core

F32 = mybir.dt.float32
BF16 = mybir.dt.bfloat16

GROUPS = 4          # chunk groups per tile (8 r32 chunks each)
GSZ = 32 // GROUPS


def _hadamard(n: int) -> np.ndarray:
    h = np.array([[1.0]], dtype=np.float32)
    while h.shape[0] < n:
        h = np.block([[h, h], [h, -h]])
    assert h.shape[0] == n
    return h.astype(np.float32)


def _build_nc():
    import ml_dtypes

    W1 = np.kron(np.eye(4, dtype=np.float32), _hadamard(32))   # [128,128]
    H128 = _hadamard(128)                                      # [128,128]

    nc = bacc.Bacc("TRN2", target_bir_lowering=False, debug=False,
                   num_devices=N_CORES)

    x = nc.dram_tensor("x", [NT, 128, 4096], BF16, kind="ExternalInput")
    y = nc.dram_tensor("y", [NT, 128, 4096], BF16, kind="ExternalOutput")

    w1_d = nc.inline_tensor(W1.astype(ml_dtypes.bfloat16), "w1c")
    h128_d = nc.inline_tensor(H128.astype(ml_dtypes.bfloat16), "h128c")

    with tile.TileContext(nc) as tc:
        with (
            tc.tile_pool(name="wpool", bufs=1) as wpool,
            tc.tile_pool(name="inp", bufs=6) as inp,
            tc.tile_pool(name="ps1p", bufs=2, space="PSUM") as ps1p,
            tc.tile_pool(name="zbp", bufs=3) as zbp,
            tc.tile_pool(name="ps2p", bufs=2, space="PSUM") as ps2p,
            tc.tile_pool(name="outp", bufs=6) as outp,
        ):
            w1 = wpool.tile([128, 128], BF16, tag="w1")
            h128 = wpool.tile([128, 128], BF16, tag="h128")

            for t in range(NT):
                # Half-tile pool objects: dependencies are half-tile
                # granular, so group 0 compute starts after the first
                # half-load and each store waits only on its own half.
                tins = []
                for h in range(2):
                    th = inp.tile([128, 2048], BF16, tag=f"tin{h}")
                    nc.gpsimd.dma_start(th[:], x[t][:, h * 2048:(h + 1) * 2048])
                    tins.append(th)
                if t == 0:
                    # Issue after the first loads so the pipeline fill
                    # leads the program.
                    nc.sync.dma_start(w1[:], w1_d[:])
                    nc.sync.dma_start(h128[:], h128_d[:])
                tout0 = outp.tile([128, 2048], BF16, tag="tout0")
                tout1 = outp.tile([128, 2048], BF16, tag="tout1")
                touts = [tout0, tout1]
                for g in range(GROUPS):
                    half = g // 2
                    tin = tins[half]
                    tout = touts[half]
                    ps1 = ps1p.tile([128, GSZ * 128], F32, tag="ps1")
                    for k in range(GSZ):
                        r32 = (g % 2) * GSZ + k
                        nc.tensor.matmul(
                            ps1[:, k * 128:(k + 1) * 128],
                            tin[:, r32 * 128:(r32 + 1) * 128],
                            w1[:], start=True, stop=True)
                    zb = zbp.tile([128, GSZ * 128], BF16, tag="zb")
                    nc.vector.tensor_copy(zb[:], ps1[:])
                    ps2 = ps2p.tile([128, GSZ * 128], F32, tag="ps2")
                    for m in range(2):
                        nc.tensor.matmul(
                            ps2[:, m * 512:(m + 1) * 512],
                            h128[:],
                            zb[:, m * 512:(m + 1) * 512],
                            start=True, stop=True)
                    nc.scalar.activation(
                        tout[:, (g % 2) * GSZ * 128:((g % 2) + 1) * GSZ * 128],
                        ps2[:],
                        mybir.ActivationFunctionType.Prelu,
                        bias=0.0, scale=1.0 / 64.0, alpha=0.1)
                    if g % 2 == 1:
                        nc.sync.dma_start(
                            y[t][:, half * 2048:(half + 1) * 2048],
                            tout[:])
    nc.finalize()
    return nc


_NC_CACHE = {}


def _get_nc():
    if "nc" not in _NC_CACHE:
        _NC_CACHE["nc"] = _build_nc()
    return _NC_CACHE["nc"]


def run(x: np.ndarray, trace: bool = False):
    """Returns (y, BassKernelResults)."""
    import ml_dtypes

    x = np.ascontiguousarray(x, dtype=np.float32)
    xb = x.reshape(-1, D).astype(ml_dtypes.bfloat16)
    shards = []
    for c in range(N_CORES):
        v = xb[c * ROWS_PER_CORE:(c + 1) * ROWS_PER_CORE]
        # rows [t, r4, r32, J, c] -> [t, (r4 J), (r32 c)]
        v = v.reshape(NT, 4, 32, 32, 128).transpose(0, 1, 3, 2, 4)
        shards.append(np.ascontiguousarray(v).reshape(NT, 128, 4096))
    nc = _get_nc()
    res = run_bass_kernel_spmd(
        nc, [{"x": s} for s in shards], core_ids=list(range(N_CORES)),
        trace=trace)
    outs = []
    for r in res.results:
        # y_dev [t, c', (r32, r4, j')] -> rows [t, r4, r32, j', c'].
        # Permute in bf16 (dtype-agnostic strided copy), then upcast
        # contiguously — astype on a strided bf16 view is very slow.
        v = np.asarray(r["y"]).reshape(NT, 128, 32, 4, 32)
        v = np.ascontiguousarray(v.transpose(0, 3, 2, 4, 1))
        outs.append(v.astype(np.float32).reshape(ROWS_PER_CORE, D))
    out = np.concatenate(outs, axis=0)
    return out.reshape(x.shape), res


def kernel(x: np.ndarray) -> np.ndarray:
    out, _ = run(x, trace=False)
    return out
